# revision 24
# baseline (speedup 1.0000x reference)
"""Trainium2 Bass kernel for nn_CopyGenerator (scatter_memory).

Computation (see the reference):
  out_tgt = log_softmax(hidden @ W.T + b)                    [T,B,VT]
  gate1m  = 1 - sigmoid(dec @ Wc.T + bc)                     [T,B]
  ext[t,b,v] = gate1m[t,b] * sum_s attn[t,b,s]*(idx[s,b]==v), idx==UNK masked
  out_ext = log(clip(ext, 1e-3, 1-1e-3))                     [T,B,VE]
  out = concat([out_tgt, out_ext], -1)

Sharding (8 cores):
  - Big matmul + log_softmax: column-parallel over tgt vocab (each core owns a
    4000-wide W slice, SBUF-resident bf16; all 3200 rows). The softmax
    denominator needs the full-vocab sum -> per-chunk partial row sums are
    AllGathered across cores (cheaper than AllReduce in latency) and summed
    locally; pipelined collectives, one per row-chunk.
  - Copy-gate + scatter-add over ext vocab: data-parallel over batch (8 batch
    elements per core). The scatter-add is aw.T @ onehot(idx) on the tensor
    engine (exact, handles duplicate indices); attn is fed as a bf16 hi/lo
    pair so the ext sums are fp32-accurate. Batch elements are processed in
    pairs packed into 64-row psum blocks (rows 50..63 are zero padding) so
    the Ln/clamp passes cover two batch elements per op. Ext pairs are
    interleaved between row-chunks so they hide in the collective latency.

Performance structure:
  - outputs are written bf16 and widened to f32 on the host (halves out DMA).
  - psum drains are 2 big strided DVE copies per tile; the final
    out = x - ln(tot) runs in-place on DVE in the packed-bf16 fast mode.
  - chunk g's post-collective output work is embedded into chunk g+1's
    emission at dependency-matched positions so no engine queue stalls.
  - DMA issues are spread across sequencers: loads on SP, collective staging
    on ACT/SP, output stores on Pool.
"""

import sys

if "/opt/trn_rl_repo" not in sys.path:
    sys.path.insert(0, "/opt/trn_rl_repo")

from contextlib import ExitStack

import ml_dtypes
import numpy as np

import concourse.bass as bass
import concourse.mybir as mybir
import concourse.tile as tile
from concourse import bacc
from concourse.bass_utils import run_bass_kernel_spmd

F32 = mybir.dt.float32
BF16 = mybir.dt.bfloat16
I16 = mybir.dt.int16
AF = mybir.ActivationFunctionType
OP = mybir.AluOpType

T, B, S, H = 50, 64, 100, 512
VT, VE = 32000, 5000
N_CORES = 8
VL = VT // N_CORES       # 4000 vocab cols per core
BL = B // N_CORES        # 8 batch per core (ext part)
R = T * B                # 3200 rows
BP = 64                  # padded rows per batch element (T=50 -> 64)
RL = BL * BP             # 512 padded gate/attn cols per core
KT = H // 128            # 4 k-tiles
MT = R // 128            # 25 m-tiles
CHS = [4, 6, 6, 6, 3]    # m-tiles per lse chunk: small first chunk primes the
                         # x-recycle pipeline, small last chunk = short tail
NCH = len(CHS)
CST = [sum(CHS[:i]) for i in range(NCH)]   # chunk start tile
NW = 500                 # matmul n-block width (<=512 f32 per psum bank)
NP = BL // 2             # 4 ext batch pairs

LOG_LO = float(np.log(0.001))
LOG_HI = float(np.log(1.0 - 0.001))

_CACHE = {}


def _dedupe_act_table_loads(nc):
    """Collapse activation-table thrash: point every load at a table that
    serves its following activations when one exists, then drop loads that
    re-load the already-loaded table. Saves ~1.8us per removed load on ACT."""
    from concourse.hw_specs import get_activation_tables
    tables = list(get_activation_tables(nc.m.arch).items())
    for blk in nc.m.functions[0].blocks:
        insts = blk.instructions
        loads = [(i, inst) for i, inst in enumerate(insts)
                 if isinstance(inst, mybir.InstLoadActFuncSet)]
        if not loads:
            continue
        for li, (pos, inst) in enumerate(loads):
            end = loads[li + 1][0] if li + 1 < len(loads) else len(insts)
            funcs = {s.func for s in insts[pos:end]
                     if isinstance(s, mybir.InstActivation)}
            if not funcs:
                continue
            want = funcs | {AF.Exp, AF.Ln, AF.Identity, AF.Copy}
            pick = None
            for tid, (name, fs) in enumerate(tables):
                if want <= fs:
                    pick = tid
                    break
            if pick is None:
                for tid, (name, fs) in enumerate(tables):
                    if funcs <= fs:
                        pick = tid
                        break
            if pick is not None:
                inst.act_func_set_id = pick
        cur = None
        to_drop = []
        for pos, inst in loads:
            if cur is not None and inst.act_func_set_id == cur:
                si = inst.sync_info
                clean = si is None or (not si.on_wait and not si.on_update)
                if clean:
                    to_drop.append(inst)
                    continue
            cur = inst.act_func_set_id
        for inst in to_drop:
            insts.remove(inst)


def _build(with_bias):
    nc = bacc.Bacc("TRN2", target_bir_lowering=False, debug=False,
                   num_devices=N_CORES)

    hT = [nc.dram_tensor(f"hT{k}", [128, R], BF16, kind="ExternalInput").ap()
          for k in range(KT)]
    wT = [nc.dram_tensor(f"wT{k}", [128, VL], BF16, kind="ExternalInput").ap()
          for k in range(KT)]
    if with_bias:
        brow = nc.dram_tensor("brow", [1, VL], BF16, kind="ExternalInput").ap()
    dT = nc.dram_tensor("dT", [KT, 128, RL], F32, kind="ExternalInput").ap()
    wcT = nc.dram_tensor("wcT", [KT, 128, 1], F32, kind="ExternalInput").ap()
    bc_t = nc.dram_tensor("bc", [1, 1], F32, kind="ExternalInput").ap()
    # attn hi/lo bf16 split, 64-padded b-major cols: [2, S, BL*64]
    attnT = nc.dram_tensor("attnT", [2, S, RL], BF16, kind="ExternalInput").ap()
    idx_t = nc.dram_tensor("idx", [S, BL], F32, kind="ExternalInput").ap()

    out_tgt = nc.dram_tensor("out_tgt", [R, VL], BF16, kind="ExternalOutput").ap()
    out_ext = nc.dram_tensor("out_ext", [BL, T, VE], BF16, kind="ExternalOutput").ap()

    # per-chunk partial sums travel as 4 un-folded accumulator columns per
    # tile (one per psum quarter); folded only after the all-gather.
    cc_in = [nc.dram_tensor(f"cc_in{g}", [128, 4 * CHS[g]], F32).ap()
             for g in range(NCH)]
    cc_out = [nc.dram_tensor(f"cc_out{g}", [N_CORES, 128, 4 * CHS[g]], F32,
                             addr_space="Shared").ap()
              for g in range(NCH)]

    core_ids = list(range(N_CORES))

    with tile.TileContext(nc) as tc, ExitStack() as ctx:
        const = ctx.enter_context(tc.tile_pool(name="const", bufs=1))
        xpool = ctx.enter_context(tc.tile_pool(name="x", bufs=10))
        scr = ctx.enter_context(tc.tile_pool(name="scr", bufs=3))
        stpool = ctx.enter_context(tc.tile_pool(name="st", bufs=2))
        statpool = ctx.enter_context(tc.tile_pool(name="stat", bufs=2))
        gathpool = ctx.enter_context(tc.tile_pool(name="gath", bufs=2))
        lnpool = ctx.enter_context(tc.tile_pool(name="lng", bufs=2))
        pspool = ctx.enter_context(tc.tile_pool(name="ps", bufs=2, space="PSUM"))

        # ---- persistent SBUF loads: matmul k-tiles first (they gate PE),
        # small gate/ext inputs after ----
        hT_sb = [const.tile([128, R], BF16, name=f"hts{k}") for k in range(KT)]
        wT_sb = [const.tile([128, VL], BF16, name=f"wts{k}") for k in range(KT)]
        # split loads: the pieces the first tiles touch come first, so PE
        # starts ~3us in instead of waiting for the full 8MB of W+h
        HC0 = CHS[0] * 128
        for k in range(KT):
            nc.sync.dma_start(wT_sb[k][:, :4 * NW], wT[k][:, :4 * NW])
            nc.sync.dma_start(hT_sb[k][:, :HC0], hT[k][:, :HC0])
        for k in range(KT):
            nc.sync.dma_start(wT_sb[k][:, 4 * NW:], wT[k][:, 4 * NW:])
        for k in range(KT):
            nc.sync.dma_start(hT_sb[k][:, HC0:], hT[k][:, HC0:])
        dT_sb = const.tile([128, KT * RL], F32)
        for k in range(KT):
            nc.sync.dma_start(dT_sb[:, k * RL:(k + 1) * RL], dT[k])
        wcT_sb = const.tile([128, KT], F32)
        for k in range(KT):
            nc.sync.dma_start(wcT_sb[:, k:k + 1], wcT[k])
        bc_sb = const.tile([1, 1], F32)
        nc.sync.dma_start(bc_sb[:], bc_t[:])
        attnT_sb = const.tile([S, 2 * RL], BF16)
        nc.sync.dma_start(attnT_sb[:, :RL], attnT[0])
        nc.sync.dma_start(attnT_sb[:, RL:], attnT[1])
        idx_sb = const.tile([S, BL], F32)
        nc.sync.dma_start(idx_sb[:], idx_t[:])
        if with_bias:
            b_sb = const.tile([1, VL], BF16)
            nc.sync.dma_start(b_sb[:], brow[:])
            ones_sb = const.tile([1, 128], BF16)
            nc.vector.memset(ones_sb[:], 1.0)
        onesT = const.tile([1, 64], F32)
        nc.vector.memset(onesT[:], 1.0)
        iota_sb = const.tile([S, VE], I16)
        nc.gpsimd.iota(iota_sb[:], pattern=[[1, VE]], base=0, channel_multiplier=0,
                       allow_small_or_imprecise_dtypes=True)
        # kill ext-vocab column 0 (UNK): make it unmatchable
        nc.gpsimd.memset(iota_sb[:, 0:1], -1.0)

        sig = const.tile([128, NP], F32)
        g1m = const.tile([128, NP], F32)

        def emit_gate():
            # g1m[t, b] = 1 - sigmoid(dec[t,b] . Wc + bc); dT cols are
            # 64-padded b-major; pairs pack to psum partition halves.
            gp = pspool.tile([128, 2048], F32, tag="ps")
            for b in range(BL):
                p, half = b // 2, (b % 2) * BP
                for k in range(KT):
                    lhs = dT_sb[:, k * RL + b * BP: k * RL + (b + 1) * BP]
                    nc.tensor.matmul(gp[half:half + BP, p:p + 1], lhsT=lhs,
                                     rhs=wcT_sb[:, k:k + 1],
                                     start=(k == 0), stop=False)
                nc.tensor.matmul(gp[half:half + BP, p:p + 1], lhsT=onesT[:],
                                 rhs=bc_sb[:], start=False, stop=True)
            nc.scalar.activation(sig[:], gp[:, :NP], AF.Sigmoid)
            nc.vector.tensor_scalar(g1m[:], sig[:], -1.0, 1.0, OP.mult, OP.add)

        # ---- main: logits, online logsumexp, pipelined output ----
        gaths = [None] * NCH
        xs = [[None] * CHS[g] for g in range(NCH)]
        lngs = [None] * NCH

        def emit_out_begin(g):
            # local sum of the gathered per-core partials (still 4 columns
            # per tile), fold the quarters, then ln
            sz = 4 * CHS[g]
            gath_g = gaths[g]
            tot_g = statpool.tile([128, 4 * max(CHS)], F32, tag="tot")
            nc.vector.tensor_tensor(tot_g[:, :sz], gath_g[:, 0:sz],
                                    gath_g[:, sz:2 * sz], OP.add)
            for rr in range(2, N_CORES):
                nc.vector.tensor_tensor(tot_g[:, :sz], tot_g[:, :sz],
                                        gath_g[:, rr * sz:(rr + 1) * sz],
                                        OP.add)
            fold = statpool.tile([128, max(CHS)], F32, tag="fold")
            nc.vector.tensor_reduce(
                fold[:, :CHS[g]],
                tot_g[:, :sz].rearrange("p (j q) -> p j q", q=4),
                mybir.AxisListType.X, OP.add)
            lng = lnpool.tile([128, max(CHS)], F32, tag="lng")
            lngs[g] = lng
            nc.scalar.activation(lng[:, :CHS[g]], fold[:, :CHS[g]], AF.Ln)

        def emit_out_one(g, j, split=1):
            # out = x - ln(tot) in-place on DVE (packed-bf16 fast mode),
            # store from the Pool queue. split>1 pipelines the store behind
            # the subtract in column pieces and issues from the idle SP
            # queue (used for the tail-critical last chunk).
            m = CST[g] + j
            x_m = xs[g][j]
            w = VL // split
            for s in range(split):
                nc.vector.tensor_scalar(x_m[:, s * w:(s + 1) * w],
                                        x_m[:, s * w:(s + 1) * w],
                                        lngs[g][:, j:j + 1],
                                        None, OP.subtract)
                eng = nc.sync if split > 1 else nc.gpsimd
                eng.dma_start(
                    out_tgt[m * 128:(m + 1) * 128, s * w:(s + 1) * w],
                    x_m[:, s * w:(s + 1) * w])

        ext_ohs = [None] * NP

        def emit_oh(p, bb):
            # one-hot prefetch for ext pair p (DVE int16 iota, 4x mode)
            if ext_ohs[p] is None:
                ext_ohs[p] = [None, None]
            oh = scr.tile([S, VE], BF16, tag="scr")
            nc.vector.tensor_scalar(oh[:], iota_sb[:],
                                    idx_sb[:, 2 * p + bb:2 * p + bb + 1],
                                    None, OP.is_equal)
            ext_ohs[p][bb] = oh

        def emit_chunk(g):
            sz = CHS[g]
            sums_g = statpool.tile([128, 4 * max(CHS)], F32, tag="sums")
            oh_at = {min(2, sz - 2): 0, min(3, sz - 1): 1}
            for j in range(sz):
                m = CST[g] + j
                x_m = xpool.tile([128, VL], BF16, tag="x")
                xs[g][j] = x_m
                E = scr.tile([128, VE], BF16, tag="scr")
                for half in range(2):
                    ps = pspool.tile([128, 2048], F32, tag="ps")
                    for k in range(KT):
                        for q in range(4):
                            n = half * 4 + q
                            nc.tensor.matmul(
                                ps[:, q * 512: q * 512 + NW],
                                lhsT=hT_sb[k][:, m * 128:(m + 1) * 128],
                                rhs=wT_sb[k][:, n * NW:(n + 1) * NW],
                                start=(k == 0),
                                stop=(k == KT - 1) and not with_bias)
                    if with_bias:
                        for q in range(4):
                            n = half * 4 + q
                            nc.tensor.matmul(
                                ps[:, q * 512: q * 512 + NW],
                                lhsT=ones_sb[:],
                                rhs=b_sb[:, n * NW:(n + 1) * NW],
                                start=False, stop=True)
                    # exp straight from psum (ACT), accumulating the row sums
                    # of this psum quarter-pair: the collective input no
                    # longer waits on the DVE drain queue
                    for q2 in range(2):
                        qq = half * 2 + q2
                        esrc = ps[:, q2 * 1024:(q2 + 1) * 1024].rearrange(
                            "p (b n) -> p b n", b=2)[:, :, :NW]
                        edst = E[:, qq * 1000:(qq + 1) * 1000].rearrange(
                            "p (b n) -> p b n", b=2)
                        nc.scalar.activation(
                            edst, esrc, AF.Exp,
                            accum_out=sums_g[:, 4 * j + qq:4 * j + qq + 1])
                    src = ps[:].rearrange("p (b n) -> p b n", b=4)[:, :, :NW]
                    dst = x_m[:, half * 4 * NW:(half + 1) * 4 * NW].rearrange(
                        "p (b n) -> p b n", b=4)
                    nc.vector.tensor_copy(dst, src)
                    # prefetch next ext pair's one-hots mid-chunk so the ext
                    # matmuls never stall the in-order PE queue
                    if g >= 1 and half == 0 and j in oh_at:
                        emit_oh(g - 1, oh_at[j])
                if j == min(2, sz - 1) and g >= 1:
                    # the previous chunk's gathered sums are back by now
                    emit_out_begin(g - 1)
            # collective staging: store partial sums (ACT just produced them),
            # all-gather on pool, load all per-core blocks back in one DMA
            # (SBUF side stays partition-first; the DRAM side is permuted).
            nc.scalar.dma_start(cc_in[g][:], sums_g[:, :4 * sz])
            nc.gpsimd.collective_compute(
                "AllGather", OP.bypass,
                replica_groups=[core_ids],
                ins=[cc_in[g][:]], outs=[cc_out[g][:]])
            gath_g = gathpool.tile([128, N_CORES * 4 * max(CHS)], F32,
                                   tag="gath")
            gaths[g] = gath_g
            nc.sync.dma_start(
                gath_g[:, :N_CORES * 4 * sz].rearrange(
                    "p (r c) -> p r c", r=N_CORES),
                cc_out[g].rearrange("r p c -> p r c"))
            # chunk g-1's subtracts + stores run in the ext-pair window,
            # where DVE has slack
            if g >= 1:
                for j in range(CHS[g - 1]):
                    emit_out_one(g - 1, j)

        # ---- ext pair: scatter-add via one-hot matmul ----
        # batch pairs packed into 64-row psum halves; one-hots on DVE (int16
        # iota, 4x mode); Ln(scale=g1m) reads psum directly; clamp on DVE.
        def emit_ext_pair(p):
            ohs = ext_ohs[p]
            st = stpool.tile([128, VE], BF16, tag="st")
            for grp, gw in ((0, 4), (4, 4), (8, 2)):
                ps = pspool.tile([128, 2048], F32, tag="ps")
                for hl in range(2):
                    for bb in range(2):
                        b = 2 * p + bb
                        lhsT = attnT_sb[:, hl * RL + b * BP:
                                        hl * RL + (b + 1) * BP]
                        for q in range(gw):
                            nb = grp + q
                            nc.tensor.matmul(
                                ps[bb * BP:(bb + 1) * BP,
                                   q * 512: q * 512 + NW],
                                lhsT=lhsT,
                                rhs=ohs[bb][:, nb * NW:(nb + 1) * NW],
                                start=(hl == 0), stop=(hl == 1))
                src = ps[:].rearrange("p (b n) -> p b n", b=4)[:, :gw, :NW]
                dst = st[:, grp * NW:(grp + gw) * NW].rearrange(
                    "p (b n) -> p b n", b=gw)
                # ext = raw * g1m folded into Ln's per-partition scale;
                # clip is done in log space (Ln(0) = -inf clips to LOG_LO)
                nc.scalar.activation(dst, src, AF.Ln, scale=g1m[:, p:p + 1])
            nc.vector.tensor_scalar(st[:], st[:], LOG_LO, LOG_HI,
                                    OP.max, OP.min)
            nc.gpsimd.dma_start(out_ext[2 * p], st[:T, :])
            nc.gpsimd.dma_start(out_ext[2 * p + 1], st[BP:BP + T, :])

        for g in range(NCH):
            emit_chunk(g)
            if g == 0:
                emit_gate()
            else:
                emit_ext_pair(g - 1)
        # last chunk's output tail: half-tile pieces pipeline the final DMAs
        emit_out_begin(NCH - 1)
        for j in range(CHS[NCH - 1]):
            emit_out_one(NCH - 1, j, split=2)

    nc.compile()
    _dedupe_act_table_loads(nc)
    return nc


def _get_nc(with_bias=False):
    key = ("nc", with_bias)
    if key not in _CACHE:
        _CACHE[key] = _build(with_bias)
    return _CACHE[key]


def kernel(**inputs):
    hidden = np.asarray(inputs["hidden"], dtype=np.float32)
    dec = np.asarray(inputs["dec_rnn_output"], dtype=np.float32)
    attn = np.asarray(inputs["attn"], dtype=np.float32)
    c2e = np.asarray(inputs["copy_to_ext"])
    W = np.asarray(inputs["W"], dtype=np.float32)
    bvec = np.asarray(inputs["b"], dtype=np.float32)
    Wc = np.asarray(inputs["Wc"], dtype=np.float32)
    bc = np.asarray(inputs["bc"], dtype=np.float32)

    with_bias = bool(np.any(bvec))
    bf = ml_dtypes.bfloat16
    hT_np = np.ascontiguousarray(
        hidden.reshape(R, H).T.reshape(KT, 128, R)).astype(bf)
    wcT_np = np.ascontiguousarray(Wc.reshape(1, H).T.reshape(KT, 128, 1))
    bc_np = bc.reshape(1, 1)

    in_maps = []
    for c in range(N_CORES):
        vs = slice(c * VL, (c + 1) * VL)
        bs = slice(c * BL, (c + 1) * BL)
        wT_np = np.ascontiguousarray(W[vs].T.reshape(KT, 128, VL)).astype(bf)
        # dT cols are 64-padded b-major: col b*64 + t
        dpad = np.zeros((BL, BP, H), np.float32)
        dpad[:, :T] = dec[:, bs, :].transpose(1, 0, 2)
        dT_np = np.ascontiguousarray(
            dpad.reshape(RL, H).T.reshape(KT, 128, RL))
        # attnT[s, b*64 + t] = attn[t, c*BL+b, s]; hi/lo bf16 split
        apad = np.zeros((S, BL, BP), np.float32)
        apad[:, :, :T] = attn[:, bs, :].transpose(2, 1, 0)
        at = apad.reshape(S, RL)
        at_hi = at.astype(bf)
        at_lo = (at - at_hi.astype(np.float32)).astype(bf)
        attnT_np = np.ascontiguousarray(np.stack([at_hi, at_lo]))
        idx_np = np.ascontiguousarray(c2e[:, bs]).astype(np.float32)
        m = {"dT": dT_np, "wcT": wcT_np, "bc": bc_np,
             "attnT": attnT_np, "idx": idx_np}
        for k in range(KT):
            m[f"hT{k}"] = np.ascontiguousarray(hT_np[k])
            m[f"wT{k}"] = np.ascontiguousarray(wT_np[k])
        if with_bias:
            m["brow"] = bvec[vs].reshape(1, VL).astype(bf)
        in_maps.append(m)

    nc = _get_nc(with_bias)
    res = run_bass_kernel_spmd(nc, in_maps, core_ids=list(range(N_CORES)))

    out = np.empty((T, B, VT + VE), dtype=np.float32)
    for c in range(N_CORES):
        r = res.results[c]
        out[:, :, c * VL:(c + 1) * VL] = np.asarray(
            r["out_tgt"], dtype=np.float32).reshape(T, B, VL)
        out[:, c * BL:(c + 1) * BL, VT:] = np.asarray(
            r["out_ext"], dtype=np.float32).transpose(1, 0, 2)
    return out


# revision 26
# speedup vs baseline: 1.1767x; 1.1767x over previous
"""Trainium2 Bass kernel for nn_CopyGenerator (scatter_memory).

Computation (see the reference):
  out_tgt = log_softmax(hidden @ W.T + b)                    [T,B,VT]
  gate1m  = 1 - sigmoid(dec @ Wc.T + bc)                     [T,B]
  ext[t,b,v] = gate1m[t,b] * sum_s attn[t,b,s]*(idx[s,b]==v), idx==UNK masked
  out_ext = log(clip(ext, 1e-3, 1-1e-3))                     [T,B,VE]
  out = concat([out_tgt, out_ext], -1)

Sharding (8 cores):
  - Big matmul + log_softmax: column-parallel over tgt vocab (each core owns a
    4000-wide W slice, SBUF-resident bf16; all 3200 rows). The softmax
    denominator needs the full-vocab sum -> per-chunk partial row sums are
    AllGathered across cores (cheaper than AllReduce in latency) and summed
    locally; pipelined collectives, one per row-chunk.
  - Copy-gate + scatter-add over ext vocab: data-parallel over batch (8 batch
    elements per core). The scatter-add is aw.T @ onehot(idx) on the tensor
    engine (exact, handles duplicate indices); attn is fed as a bf16 hi/lo
    pair so the ext sums are fp32-accurate. Batch elements are processed in
    pairs packed into 64-row psum blocks (rows 50..63 are zero padding) so
    the Ln/clamp passes cover two batch elements per op. Ext pairs are
    interleaved between row-chunks so they hide in the collective latency.

Performance structure:
  - outputs are written bf16 and widened to f32 on the host (halves out DMA).
  - psum drains are 2 big strided DVE copies per tile; the final
    out = x - ln(tot) runs in-place on DVE in the packed-bf16 fast mode.
  - chunk g's post-collective output work is embedded into chunk g+1's
    emission at dependency-matched positions so no engine queue stalls.
  - DMA issues are spread across sequencers: loads on SP, collective staging
    on ACT/SP, output stores on Pool.
"""

import sys

if "/opt/trn_rl_repo" not in sys.path:
    sys.path.insert(0, "/opt/trn_rl_repo")

from contextlib import ExitStack

import ml_dtypes
import numpy as np

import concourse.bass as bass
import concourse.mybir as mybir
import concourse.tile as tile
from concourse import bacc
from concourse.bass_utils import run_bass_kernel_spmd

F32 = mybir.dt.float32
BF16 = mybir.dt.bfloat16
I16 = mybir.dt.int16
AF = mybir.ActivationFunctionType
OP = mybir.AluOpType

T, B, S, H = 50, 64, 100, 512
VT, VE = 32000, 5000
N_CORES = 8
VL = VT // N_CORES       # 4000 vocab cols per core
BL = B // N_CORES        # 8 batch per core (ext part)
R = T * B                # 3200 rows
BP = 64                  # padded rows per batch element (T=50 -> 64)
RL = BL * BP             # 512 padded gate/attn cols per core
KT = H // 128            # 4 k-tiles
MT = R // 128            # 25 m-tiles
CHS = [4, 6, 6, 6, 3]    # m-tiles per lse chunk: small first chunk primes the
                         # x-recycle pipeline, small last chunk = short tail
NCH = len(CHS)
CST = [sum(CHS[:i]) for i in range(NCH)]   # chunk start tile
NW = 500                 # matmul n-block width (<=512 f32 per psum bank)
NP = BL // 2             # 4 ext batch pairs

LOG_LO = float(np.log(0.001))
LOG_HI = float(np.log(1.0 - 0.001))

_CACHE = {}


def _dedupe_act_table_loads(nc):
    """Collapse activation-table thrash: point every load at a table that
    serves its following activations when one exists, then drop loads that
    re-load the already-loaded table. Saves ~1.8us per removed load on ACT."""
    from concourse.hw_specs import get_activation_tables
    tables = list(get_activation_tables(nc.m.arch).items())
    for blk in nc.m.functions[0].blocks:
        insts = blk.instructions
        loads = [(i, inst) for i, inst in enumerate(insts)
                 if isinstance(inst, mybir.InstLoadActFuncSet)]
        if not loads:
            continue
        for li, (pos, inst) in enumerate(loads):
            end = loads[li + 1][0] if li + 1 < len(loads) else len(insts)
            funcs = {s.func for s in insts[pos:end]
                     if isinstance(s, mybir.InstActivation)}
            if not funcs:
                continue
            want = funcs | {AF.Exp, AF.Ln, AF.Identity, AF.Copy}
            pick = None
            for tid, (name, fs) in enumerate(tables):
                if want <= fs:
                    pick = tid
                    break
            if pick is None:
                for tid, (name, fs) in enumerate(tables):
                    if funcs <= fs:
                        pick = tid
                        break
            if pick is not None:
                inst.act_func_set_id = pick
        cur = None
        to_drop = []
        for pos, inst in loads:
            if cur is not None and inst.act_func_set_id == cur:
                si = inst.sync_info
                clean = si is None or (not si.on_wait and not si.on_update)
                if clean:
                    to_drop.append(inst)
                    continue
            cur = inst.act_func_set_id
        for inst in to_drop:
            insts.remove(inst)


def _build(with_bias):
    nc = bacc.Bacc("TRN2", target_bir_lowering=False, debug=False,
                   num_devices=N_CORES)

    hT = [nc.dram_tensor(f"hT{k}", [128, R], BF16, kind="ExternalInput").ap()
          for k in range(KT)]
    wT = [nc.dram_tensor(f"wT{k}", [128, VL], BF16, kind="ExternalInput").ap()
          for k in range(KT)]
    if with_bias:
        brow = nc.dram_tensor("brow", [1, VL], BF16, kind="ExternalInput").ap()
    dT = nc.dram_tensor("dT", [KT, 128, RL], F32, kind="ExternalInput").ap()
    wcT = nc.dram_tensor("wcT", [KT, 128, 1], F32, kind="ExternalInput").ap()
    bc_t = nc.dram_tensor("bc", [1, 1], F32, kind="ExternalInput").ap()
    # attn hi/lo bf16 split, 64-padded b-major cols: [2, S, BL*64]
    attnT = nc.dram_tensor("attnT", [2, S, RL], BF16, kind="ExternalInput").ap()
    idx_t = nc.dram_tensor("idx", [S, BL], F32, kind="ExternalInput").ap()

    out_tgt = nc.dram_tensor("out_tgt", [R, VL], BF16, kind="ExternalOutput").ap()
    out_ext = nc.dram_tensor("out_ext", [BL, T, VE], BF16, kind="ExternalOutput").ap()

    # per-chunk partial sums travel as 4 un-folded accumulator columns per
    # tile (one per psum quarter); folded only after the all-gather.
    cc_in = [nc.dram_tensor(f"cc_in{g}", [128, 2 * CHS[g]], F32).ap()
             for g in range(NCH)]
    cc_out = [nc.dram_tensor(f"cc_out{g}", [N_CORES, 128, 2 * CHS[g]], F32,
                             addr_space="Shared").ap()
              for g in range(NCH)]

    core_ids = list(range(N_CORES))

    with tile.TileContext(nc) as tc, ExitStack() as ctx:
        const = ctx.enter_context(tc.tile_pool(name="const", bufs=1))
        xpool = ctx.enter_context(tc.tile_pool(name="x", bufs=10))
        scr = ctx.enter_context(tc.tile_pool(name="scr", bufs=3))
        stpool = ctx.enter_context(tc.tile_pool(name="st", bufs=2))
        statpool = ctx.enter_context(tc.tile_pool(name="stat", bufs=2))
        gathpool = ctx.enter_context(tc.tile_pool(name="gath", bufs=2))
        lnpool = ctx.enter_context(tc.tile_pool(name="lng", bufs=2))
        pspool = ctx.enter_context(tc.tile_pool(name="ps", bufs=2, space="PSUM"))

        # ---- persistent SBUF loads: matmul k-tiles first (they gate PE),
        # small gate/ext inputs after ----
        hT_sb = [const.tile([128, R], BF16, name=f"hts{k}") for k in range(KT)]
        wT_sb = [const.tile([128, VL], BF16, name=f"wts{k}") for k in range(KT)]
        # split loads: the pieces the first tiles touch come first, so PE
        # starts ~3us in instead of waiting for the full 8MB of W+h
        HC0 = CHS[0] * 128
        for k in range(KT):
            nc.sync.dma_start(wT_sb[k][:, :4 * NW], wT[k][:, :4 * NW])
            nc.sync.dma_start(hT_sb[k][:, :HC0], hT[k][:, :HC0])
        for k in range(KT):
            nc.sync.dma_start(wT_sb[k][:, 4 * NW:], wT[k][:, 4 * NW:])
        for k in range(KT):
            nc.sync.dma_start(hT_sb[k][:, HC0:], hT[k][:, HC0:])
        dT_sb = const.tile([128, KT * RL], F32)
        for k in range(KT):
            nc.sync.dma_start(dT_sb[:, k * RL:(k + 1) * RL], dT[k])
        wcT_sb = const.tile([128, KT], F32)
        for k in range(KT):
            nc.sync.dma_start(wcT_sb[:, k:k + 1], wcT[k])
        bc_sb = const.tile([1, 1], F32)
        nc.sync.dma_start(bc_sb[:], bc_t[:])
        attnT_sb = const.tile([S, 2 * RL], BF16)
        nc.sync.dma_start(attnT_sb[:, :RL], attnT[0])
        nc.sync.dma_start(attnT_sb[:, RL:], attnT[1])
        idx_sb = const.tile([S, BL], F32)
        nc.sync.dma_start(idx_sb[:], idx_t[:])
        if with_bias:
            b_sb = const.tile([1, VL], BF16)
            nc.sync.dma_start(b_sb[:], brow[:])
            ones_sb = const.tile([1, 128], BF16)
            nc.vector.memset(ones_sb[:], 1.0)
        onesT = const.tile([1, 64], F32)
        nc.vector.memset(onesT[:], 1.0)
        iota_sb = const.tile([S, VE], I16)
        nc.gpsimd.iota(iota_sb[:], pattern=[[1, VE]], base=0, channel_multiplier=0,
                       allow_small_or_imprecise_dtypes=True)
        # kill ext-vocab column 0 (UNK): make it unmatchable
        nc.gpsimd.memset(iota_sb[:, 0:1], -1.0)

        sig = const.tile([128, NP], F32)
        g1m = const.tile([128, NP], F32)

        def emit_gate():
            # g1m[t, b] = 1 - sigmoid(dec[t,b] . Wc + bc); dT cols are
            # 64-padded b-major; pairs pack to psum partition halves.
            gp = pspool.tile([128, 2048], F32, tag="ps")
            for b in range(BL):
                p, half = b // 2, (b % 2) * BP
                for k in range(KT):
                    lhs = dT_sb[:, k * RL + b * BP: k * RL + (b + 1) * BP]
                    nc.tensor.matmul(gp[half:half + BP, p:p + 1], lhsT=lhs,
                                     rhs=wcT_sb[:, k:k + 1],
                                     start=(k == 0), stop=False)
                nc.tensor.matmul(gp[half:half + BP, p:p + 1], lhsT=onesT[:],
                                 rhs=bc_sb[:], start=False, stop=True)
            nc.scalar.activation(sig[:], gp[:, :NP], AF.Sigmoid)
            nc.vector.tensor_scalar(g1m[:], sig[:], -1.0, 1.0, OP.mult, OP.add)

        # ---- main: logits, online logsumexp, pipelined output ----
        gaths = [None] * NCH
        xs = [[None] * CHS[g] for g in range(NCH)]
        lngs = [None] * NCH

        def emit_out_begin(g):
            # local sum of the gathered per-core partials (still 4 columns
            # per tile), fold the quarters, then ln
            sz = 2 * CHS[g]
            gath_g = gaths[g]
            tot_g = statpool.tile([128, 2 * max(CHS)], F32, tag="tot")
            nc.vector.tensor_tensor(tot_g[:, :sz], gath_g[:, 0:sz],
                                    gath_g[:, sz:2 * sz], OP.add)
            for rr in range(2, N_CORES):
                nc.vector.tensor_tensor(tot_g[:, :sz], tot_g[:, :sz],
                                        gath_g[:, rr * sz:(rr + 1) * sz],
                                        OP.add)
            fold = statpool.tile([128, max(CHS)], F32, tag="fold")
            nc.vector.tensor_reduce(
                fold[:, :CHS[g]],
                tot_g[:, :sz].rearrange("p (j q) -> p j q", q=2),
                mybir.AxisListType.X, OP.add)
            lng = lnpool.tile([128, max(CHS)], F32, tag="lng")
            lngs[g] = lng
            nc.scalar.activation(lng[:, :CHS[g]], fold[:, :CHS[g]], AF.Ln)

        def emit_out_one(g, j, split=1):
            # out = x - ln(tot) in-place on DVE (packed-bf16 fast mode),
            # store from the Pool queue. split>1 pipelines the store behind
            # the subtract in column pieces and issues from the idle SP
            # queue (used for the tail-critical last chunk).
            m = CST[g] + j
            x_m = xs[g][j]
            w = VL // split
            for s in range(split):
                nc.vector.tensor_scalar(x_m[:, s * w:(s + 1) * w],
                                        x_m[:, s * w:(s + 1) * w],
                                        lngs[g][:, j:j + 1],
                                        None, OP.subtract)
                eng = nc.sync if split > 1 else nc.gpsimd
                eng.dma_start(
                    out_tgt[m * 128:(m + 1) * 128, s * w:(s + 1) * w],
                    x_m[:, s * w:(s + 1) * w])

        ext_ohs = [None] * NP

        def emit_oh(p, bb):
            # one-hot prefetch for ext pair p (DVE int16 iota, 4x mode)
            if ext_ohs[p] is None:
                ext_ohs[p] = [None, None]
            oh = scr.tile([S, VE], BF16, tag="scr")
            nc.vector.tensor_scalar(oh[:], iota_sb[:],
                                    idx_sb[:, 2 * p + bb:2 * p + bb + 1],
                                    None, OP.is_equal)
            ext_ohs[p][bb] = oh

        def emit_chunk(g):
            sz = CHS[g]
            sums_g = statpool.tile([128, 2 * max(CHS)], F32, tag="sums")
            oh_at = {min(2, sz - 2): 0, min(3, sz - 1): 1}
            for j in range(sz):
                m = CST[g] + j
                x_m = xpool.tile([128, VL], BF16, tag="x")
                xs[g][j] = x_m
                E = scr.tile([128, VE], BF16, tag="scr")
                for half in range(2):
                    ps = pspool.tile([128, 2048], F32, tag="ps")
                    for k in range(KT):
                        for q in range(4):
                            n = half * 4 + q
                            nc.tensor.matmul(
                                ps[:, q * 512: q * 512 + NW],
                                lhsT=hT_sb[k][:, m * 128:(m + 1) * 128],
                                rhs=wT_sb[k][:, n * NW:(n + 1) * NW],
                                start=(k == 0),
                                stop=(k == KT - 1) and not with_bias)
                    if with_bias:
                        for q in range(4):
                            n = half * 4 + q
                            nc.tensor.matmul(
                                ps[:, q * 512: q * 512 + NW],
                                lhsT=ones_sb[:],
                                rhs=b_sb[:, n * NW:(n + 1) * NW],
                                start=False, stop=True)
                    src = ps[:].rearrange("p (b n) -> p b n", b=4)[:, :, :NW]
                    dst = x_m[:, half * 4 * NW:(half + 1) * 4 * NW].rearrange(
                        "p (b n) -> p b n", b=4)
                    if j == sz - 1 and half == 0:
                        # the chunk's last h0 drain goes to ACT: it frees the
                        # psum buf the following ext matmuls need, without
                        # waiting out the DVE queue
                        nc.scalar.copy(dst, src)
                    else:
                        nc.vector.tensor_copy(dst, src)
                    # prefetch next ext pair's one-hots mid-chunk so the ext
                    # matmuls never stall the in-order PE queue
                    if g >= 1 and half == 0 and j in oh_at:
                        emit_oh(g - 1, oh_at[j])
                # exp from the drained bf16 logits, accumulating the row sum
                # (per-tile halves so the accumulate stays off the psum path)
                for half in range(2):
                    nc.scalar.activation(
                        E[:, half * 2000:half * 2000 + 2000],
                        x_m[:, half * 2000:half * 2000 + 2000], AF.Exp,
                        accum_out=sums_g[:, 2 * j + half:2 * j + half + 1])
                if j == min(2, sz - 1) and g >= 1:
                    # the previous chunk's gathered sums are back by now
                    emit_out_begin(g - 1)
            # collective staging: store partial sums (ACT just produced them),
            # all-gather on pool, load all per-core blocks back in one DMA
            # (SBUF side stays partition-first; the DRAM side is permuted).
            nc.scalar.dma_start(cc_in[g][:], sums_g[:, :2 * sz])
            nc.gpsimd.collective_compute(
                "AllGather", OP.bypass,
                replica_groups=[core_ids],
                ins=[cc_in[g][:]], outs=[cc_out[g][:]])
            gath_g = gathpool.tile([128, N_CORES * 2 * max(CHS)], F32,
                                   tag="gath")
            gaths[g] = gath_g
            nc.sync.dma_start(
                gath_g[:, :N_CORES * 2 * sz].rearrange(
                    "p (r c) -> p r c", r=N_CORES),
                cc_out[g].rearrange("r p c -> p r c"))
            # chunk g-1's subtracts + stores run in the ext-pair window,
            # where DVE has slack
            if g >= 1:
                for j in range(CHS[g - 1]):
                    emit_out_one(g - 1, j)

        # ---- ext pair: scatter-add via one-hot matmul ----
        # batch pairs packed into 64-row psum halves; one-hots on DVE (int16
        # iota, 4x mode); Ln(scale=g1m) reads psum directly; clamp on DVE.
        def emit_ext_pair(p):
            ohs = ext_ohs[p]
            st = stpool.tile([128, VE], BF16, tag="st")
            for grp, gw in ((0, 4), (4, 4), (8, 2)):
                ps = pspool.tile([128, 2048], F32, tag="ps")
                for hl in range(2):
                    for bb in range(2):
                        b = 2 * p + bb
                        lhsT = attnT_sb[:, hl * RL + b * BP:
                                        hl * RL + (b + 1) * BP]
                        for q in range(gw):
                            nb = grp + q
                            nc.tensor.matmul(
                                ps[bb * BP:(bb + 1) * BP,
                                   q * 512: q * 512 + NW],
                                lhsT=lhsT,
                                rhs=ohs[bb][:, nb * NW:(nb + 1) * NW],
                                start=(hl == 0), stop=(hl == 1))
                src = ps[:].rearrange("p (b n) -> p b n", b=4)[:, :gw, :NW]
                dst = st[:, grp * NW:(grp + gw) * NW].rearrange(
                    "p (b n) -> p b n", b=gw)
                # ext = raw * g1m folded into Ln's per-partition scale;
                # clip is done in log space (Ln(0) = -inf clips to LOG_LO)
                nc.scalar.activation(dst, src, AF.Ln, scale=g1m[:, p:p + 1])
            nc.vector.tensor_scalar(st[:], st[:], LOG_LO, LOG_HI,
                                    OP.max, OP.min)
            nc.gpsimd.dma_start(out_ext[2 * p], st[:T, :])
            nc.gpsimd.dma_start(out_ext[2 * p + 1], st[BP:BP + T, :])

        for g in range(NCH):
            emit_chunk(g)
            if g == 0:
                emit_gate()
            else:
                emit_ext_pair(g - 1)
        # last chunk's output tail: half-tile pieces pipeline the final DMAs
        emit_out_begin(NCH - 1)
        for j in range(CHS[NCH - 1]):
            emit_out_one(NCH - 1, j, split=2)

    nc.compile()
    _dedupe_act_table_loads(nc)
    return nc


def _get_nc(with_bias=False):
    key = ("nc", with_bias)
    if key not in _CACHE:
        _CACHE[key] = _build(with_bias)
    return _CACHE[key]


def kernel(**inputs):
    hidden = np.asarray(inputs["hidden"], dtype=np.float32)
    dec = np.asarray(inputs["dec_rnn_output"], dtype=np.float32)
    attn = np.asarray(inputs["attn"], dtype=np.float32)
    c2e = np.asarray(inputs["copy_to_ext"])
    W = np.asarray(inputs["W"], dtype=np.float32)
    bvec = np.asarray(inputs["b"], dtype=np.float32)
    Wc = np.asarray(inputs["Wc"], dtype=np.float32)
    bc = np.asarray(inputs["bc"], dtype=np.float32)

    with_bias = bool(np.any(bvec))
    bf = ml_dtypes.bfloat16
    hT_np = np.ascontiguousarray(
        hidden.reshape(R, H).T.reshape(KT, 128, R)).astype(bf)
    wcT_np = np.ascontiguousarray(Wc.reshape(1, H).T.reshape(KT, 128, 1))
    bc_np = bc.reshape(1, 1)

    in_maps = []
    for c in range(N_CORES):
        vs = slice(c * VL, (c + 1) * VL)
        bs = slice(c * BL, (c + 1) * BL)
        wT_np = np.ascontiguousarray(W[vs].T.reshape(KT, 128, VL)).astype(bf)
        # dT cols are 64-padded b-major: col b*64 + t
        dpad = np.zeros((BL, BP, H), np.float32)
        dpad[:, :T] = dec[:, bs, :].transpose(1, 0, 2)
        dT_np = np.ascontiguousarray(
            dpad.reshape(RL, H).T.reshape(KT, 128, RL))
        # attnT[s, b*64 + t] = attn[t, c*BL+b, s]; hi/lo bf16 split
        apad = np.zeros((S, BL, BP), np.float32)
        apad[:, :, :T] = attn[:, bs, :].transpose(2, 1, 0)
        at = apad.reshape(S, RL)
        at_hi = at.astype(bf)
        at_lo = (at - at_hi.astype(np.float32)).astype(bf)
        attnT_np = np.ascontiguousarray(np.stack([at_hi, at_lo]))
        idx_np = np.ascontiguousarray(c2e[:, bs]).astype(np.float32)
        m = {"dT": dT_np, "wcT": wcT_np, "bc": bc_np,
             "attnT": attnT_np, "idx": idx_np}
        for k in range(KT):
            m[f"hT{k}"] = np.ascontiguousarray(hT_np[k])
            m[f"wT{k}"] = np.ascontiguousarray(wT_np[k])
        if with_bias:
            m["brow"] = bvec[vs].reshape(1, VL).astype(bf)
        in_maps.append(m)

    nc = _get_nc(with_bias)
    res = run_bass_kernel_spmd(nc, in_maps, core_ids=list(range(N_CORES)))

    out = np.empty((T, B, VT + VE), dtype=np.float32)
    for c in range(N_CORES):
        r = res.results[c]
        out[:, :, c * VL:(c + 1) * VL] = np.asarray(
            r["out_tgt"], dtype=np.float32).reshape(T, B, VL)
        out[:, c * BL:(c + 1) * BL, VT:] = np.asarray(
            r["out_ext"], dtype=np.float32).transpose(1, 0, 2)
    return out


# revision 27
# speedup vs baseline: 1.2410x; 1.0546x over previous
"""Trainium2 Bass kernel for nn_CopyGenerator (scatter_memory).

Computation (see the reference):
  out_tgt = log_softmax(hidden @ W.T + b)                    [T,B,VT]
  gate1m  = 1 - sigmoid(dec @ Wc.T + bc)                     [T,B]
  ext[t,b,v] = gate1m[t,b] * sum_s attn[t,b,s]*(idx[s,b]==v), idx==UNK masked
  out_ext = log(clip(ext, 1e-3, 1-1e-3))                     [T,B,VE]
  out = concat([out_tgt, out_ext], -1)

Sharding (8 cores):
  - Big matmul + log_softmax: column-parallel over tgt vocab (each core owns a
    4000-wide W slice, SBUF-resident bf16; all 3200 rows). The softmax
    denominator needs the full-vocab sum -> per-chunk partial row sums are
    AllGathered across cores (cheaper than AllReduce in latency) and summed
    locally; pipelined collectives, one per row-chunk.
  - Copy-gate + scatter-add over ext vocab: data-parallel over batch (8 batch
    elements per core). The scatter-add is aw.T @ onehot(idx) on the tensor
    engine (exact, handles duplicate indices); attn is fed as a bf16 hi/lo
    pair so the ext sums are fp32-accurate. Batch elements are processed in
    pairs packed into 64-row psum blocks (rows 50..63 are zero padding) so
    the Ln/clamp passes cover two batch elements per op. Ext pairs are
    interleaved between row-chunks so they hide in the collective latency.

Performance structure:
  - outputs are written bf16 and widened to f32 on the host (halves out DMA).
  - psum drains are 2 big strided DVE copies per tile; the final
    out = x - ln(tot) runs in-place on DVE in the packed-bf16 fast mode.
  - chunk g's post-collective output work is embedded into chunk g+1's
    emission at dependency-matched positions so no engine queue stalls.
  - DMA issues are spread across sequencers: loads on SP, collective staging
    on ACT/SP, output stores on Pool.
"""

import sys

if "/opt/trn_rl_repo" not in sys.path:
    sys.path.insert(0, "/opt/trn_rl_repo")

from contextlib import ExitStack

import ml_dtypes
import numpy as np

import concourse.bass as bass
import concourse.mybir as mybir
import concourse.tile as tile
from concourse import bacc
from concourse.bass_utils import run_bass_kernel_spmd

F32 = mybir.dt.float32
BF16 = mybir.dt.bfloat16
I16 = mybir.dt.int16
AF = mybir.ActivationFunctionType
OP = mybir.AluOpType

T, B, S, H = 50, 64, 100, 512
VT, VE = 32000, 5000
N_CORES = 8
VL = VT // N_CORES       # 4000 vocab cols per core
BL = B // N_CORES        # 8 batch per core (ext part)
R = T * B                # 3200 rows
BP = 64                  # padded rows per batch element (T=50 -> 64)
RL = BL * BP             # 512 padded gate/attn cols per core
KT = H // 128            # 4 k-tiles
MT = R // 128            # 25 m-tiles
CHS = [4, 6, 6, 6, 3]    # m-tiles per lse chunk: small first chunk primes the
                         # x-recycle pipeline, small last chunk = short tail
NCH = len(CHS)
CST = [sum(CHS[:i]) for i in range(NCH)]   # chunk start tile
NW = 500                 # matmul n-block width (<=512 f32 per psum bank)
NP = BL // 2             # 4 ext batch pairs

LOG_LO = float(np.log(0.001))
LOG_HI = float(np.log(1.0 - 0.001))

_CACHE = {}


def _dedupe_act_table_loads(nc):
    """Collapse activation-table thrash: point every load at a table that
    serves its following activations when one exists, then drop loads that
    re-load the already-loaded table. Saves ~1.8us per removed load on ACT."""
    from concourse.hw_specs import get_activation_tables
    tables = list(get_activation_tables(nc.m.arch).items())
    for blk in nc.m.functions[0].blocks:
        insts = blk.instructions
        loads = [(i, inst) for i, inst in enumerate(insts)
                 if isinstance(inst, mybir.InstLoadActFuncSet)]
        if not loads:
            continue
        for li, (pos, inst) in enumerate(loads):
            end = loads[li + 1][0] if li + 1 < len(loads) else len(insts)
            funcs = {s.func for s in insts[pos:end]
                     if isinstance(s, mybir.InstActivation)}
            if not funcs:
                continue
            want = funcs | {AF.Exp, AF.Ln, AF.Identity, AF.Copy}
            pick = None
            for tid, (name, fs) in enumerate(tables):
                if want <= fs:
                    pick = tid
                    break
            if pick is None:
                for tid, (name, fs) in enumerate(tables):
                    if funcs <= fs:
                        pick = tid
                        break
            if pick is not None:
                inst.act_func_set_id = pick
        cur = None
        to_drop = []
        for pos, inst in loads:
            if cur is not None and inst.act_func_set_id == cur:
                si = inst.sync_info
                clean = si is None or (not si.on_wait and not si.on_update)
                if clean:
                    to_drop.append(inst)
                    continue
            cur = inst.act_func_set_id
        for inst in to_drop:
            insts.remove(inst)


def _build(with_bias):
    nc = bacc.Bacc("TRN2", target_bir_lowering=False, debug=False,
                   num_devices=N_CORES)

    hT = [nc.dram_tensor(f"hT{k}", [128, R], BF16, kind="ExternalInput").ap()
          for k in range(KT)]
    wT = [nc.dram_tensor(f"wT{k}", [128, VL], BF16, kind="ExternalInput").ap()
          for k in range(KT)]
    if with_bias:
        brow = nc.dram_tensor("brow", [1, VL], BF16, kind="ExternalInput").ap()
    dT = nc.dram_tensor("dT", [KT, 128, RL], F32, kind="ExternalInput").ap()
    wcT = nc.dram_tensor("wcT", [KT, 128, 1], F32, kind="ExternalInput").ap()
    bc_t = nc.dram_tensor("bc", [1, 1], F32, kind="ExternalInput").ap()
    # attn hi/lo bf16 split, 64-padded b-major cols: [2, S, BL*64]
    attnT = nc.dram_tensor("attnT", [2, S, RL], BF16, kind="ExternalInput").ap()
    idx_t = nc.dram_tensor("idx", [S, BL], F32, kind="ExternalInput").ap()

    out_tgt = nc.dram_tensor("out_tgt", [R, VL], BF16, kind="ExternalOutput").ap()
    out_ext = nc.dram_tensor("out_ext", [BL, T, VE], BF16, kind="ExternalOutput").ap()

    cc_in = [nc.dram_tensor(f"cc_in{g}", [128, CHS[g]], F32).ap()
             for g in range(NCH)]
    cc_out = [nc.dram_tensor(f"cc_out{g}", [N_CORES, 128, CHS[g]], F32,
                             addr_space="Shared").ap()
              for g in range(NCH)]

    core_ids = list(range(N_CORES))

    with tile.TileContext(nc) as tc, ExitStack() as ctx:
        const = ctx.enter_context(tc.tile_pool(name="const", bufs=1))
        xpool = ctx.enter_context(tc.tile_pool(name="x", bufs=10))
        scr = ctx.enter_context(tc.tile_pool(name="scr", bufs=3))
        stpool = ctx.enter_context(tc.tile_pool(name="st", bufs=2))
        statpool = ctx.enter_context(tc.tile_pool(name="stat", bufs=2))
        gathpool = ctx.enter_context(tc.tile_pool(name="gath", bufs=2))
        lnpool = ctx.enter_context(tc.tile_pool(name="lng", bufs=2))
        pspool = ctx.enter_context(tc.tile_pool(name="ps", bufs=2, space="PSUM"))

        # ---- persistent SBUF loads: matmul k-tiles first (they gate PE),
        # small gate/ext inputs after ----
        hT_sb = [const.tile([128, R], BF16, name=f"hts{k}") for k in range(KT)]
        wT_sb = [const.tile([128, VL], BF16, name=f"wts{k}") for k in range(KT)]
        # split loads: the pieces the first tiles touch come first, so PE
        # starts ~3us in instead of waiting for the full 8MB of W+h
        HC0 = CHS[0] * 128
        for k in range(KT):
            nc.sync.dma_start(wT_sb[k][:, :4 * NW], wT[k][:, :4 * NW])
            nc.sync.dma_start(hT_sb[k][:, :HC0], hT[k][:, :HC0])
        for k in range(KT):
            nc.sync.dma_start(wT_sb[k][:, 4 * NW:], wT[k][:, 4 * NW:])
        for k in range(KT):
            nc.sync.dma_start(hT_sb[k][:, HC0:], hT[k][:, HC0:])
        dT_sb = const.tile([128, KT * RL], F32)
        for k in range(KT):
            nc.sync.dma_start(dT_sb[:, k * RL:(k + 1) * RL], dT[k])
        wcT_sb = const.tile([128, KT], F32)
        for k in range(KT):
            nc.sync.dma_start(wcT_sb[:, k:k + 1], wcT[k])
        bc_sb = const.tile([1, 1], F32)
        nc.sync.dma_start(bc_sb[:], bc_t[:])
        attnT_sb = const.tile([S, 2 * RL], BF16)
        nc.sync.dma_start(attnT_sb[:, :RL], attnT[0])
        nc.sync.dma_start(attnT_sb[:, RL:], attnT[1])
        idx_sb = const.tile([S, BL], F32)
        nc.sync.dma_start(idx_sb[:], idx_t[:])
        if with_bias:
            b_sb = const.tile([1, VL], BF16)
            nc.sync.dma_start(b_sb[:], brow[:])
            ones_sb = const.tile([1, 128], BF16)
            nc.vector.memset(ones_sb[:], 1.0)
        onesT = const.tile([1, 64], F32)
        nc.vector.memset(onesT[:], 1.0)
        iota_sb = const.tile([S, VE], I16)
        nc.gpsimd.iota(iota_sb[:], pattern=[[1, VE]], base=0, channel_multiplier=0,
                       allow_small_or_imprecise_dtypes=True)
        # kill ext-vocab column 0 (UNK): make it unmatchable
        nc.gpsimd.memset(iota_sb[:, 0:1], -1.0)

        sig = const.tile([128, NP], F32)
        g1m = const.tile([128, NP], F32)

        def emit_gate():
            # g1m[t, b] = 1 - sigmoid(dec[t,b] . Wc + bc); dT cols are
            # 64-padded b-major; pairs pack to psum partition halves.
            gp = pspool.tile([128, 2048], F32, tag="ps")
            for b in range(BL):
                p, half = b // 2, (b % 2) * BP
                for k in range(KT):
                    lhs = dT_sb[:, k * RL + b * BP: k * RL + (b + 1) * BP]
                    nc.tensor.matmul(gp[half:half + BP, p:p + 1], lhsT=lhs,
                                     rhs=wcT_sb[:, k:k + 1],
                                     start=(k == 0), stop=False)
                nc.tensor.matmul(gp[half:half + BP, p:p + 1], lhsT=onesT[:],
                                 rhs=bc_sb[:], start=False, stop=True)
            nc.scalar.activation(sig[:], gp[:, :NP], AF.Sigmoid)
            nc.vector.tensor_scalar(g1m[:], sig[:], -1.0, 1.0, OP.mult, OP.add)

        # ---- main: logits, online logsumexp, pipelined output ----
        gaths = [None] * NCH
        xs = [[None] * CHS[g] for g in range(NCH)]
        lngs = [None] * NCH

        def emit_out_begin(g):
            # local sum of the gathered per-core partials (still 4 columns
            # per tile), fold the quarters, then ln
            sz = CHS[g]
            gath_g = gaths[g]
            tot_g = statpool.tile([128, max(CHS)], F32, tag="tot")
            nc.vector.tensor_tensor(tot_g[:, :sz], gath_g[:, 0:sz],
                                    gath_g[:, sz:2 * sz], OP.add)
            for rr in range(2, N_CORES):
                nc.vector.tensor_tensor(tot_g[:, :sz], tot_g[:, :sz],
                                        gath_g[:, rr * sz:(rr + 1) * sz],
                                        OP.add)
            lng = lnpool.tile([128, max(CHS)], F32, tag="lng")
            lngs[g] = lng
            nc.scalar.activation(lng[:, :CHS[g]], tot_g[:, :sz], AF.Ln)

        def emit_out_one(g, j, split=1):
            # out = x - ln(tot) in-place on DVE (packed-bf16 fast mode),
            # store from the Pool queue. split>1 pipelines the store behind
            # the subtract in column pieces and issues from the idle SP
            # queue (used for the tail-critical last chunk).
            m = CST[g] + j
            x_m = xs[g][j]
            w = VL // split
            for s in range(split):
                nc.vector.tensor_scalar(x_m[:, s * w:(s + 1) * w],
                                        x_m[:, s * w:(s + 1) * w],
                                        lngs[g][:, j:j + 1],
                                        None, OP.subtract)
                eng = nc.sync if split > 1 else nc.gpsimd
                eng.dma_start(
                    out_tgt[m * 128:(m + 1) * 128, s * w:(s + 1) * w],
                    x_m[:, s * w:(s + 1) * w])

        ext_ohs = [None] * NP

        def emit_oh(p, bb):
            # one-hot prefetch for ext pair p (DVE int16 iota, 4x mode)
            if ext_ohs[p] is None:
                ext_ohs[p] = [None, None]
            oh = scr.tile([S, VE], BF16, tag="scr")
            nc.vector.tensor_scalar(oh[:], iota_sb[:],
                                    idx_sb[:, 2 * p + bb:2 * p + bb + 1],
                                    None, OP.is_equal)
            ext_ohs[p][bb] = oh

        def emit_chunk(g):
            sz = CHS[g]
            sums_g = statpool.tile([128, max(CHS)], F32, tag="sums")
            oh_at = {min(2, sz - 2): 0, min(3, sz - 1): 1}
            for j in range(sz):
                m = CST[g] + j
                x_m = xpool.tile([128, VL], BF16, tag="x")
                xs[g][j] = x_m
                E = scr.tile([128, VE], BF16, tag="scr")
                for half in range(2):
                    ps = pspool.tile([128, 2048], F32, tag="ps")
                    for k in range(KT):
                        for q in range(4):
                            n = half * 4 + q
                            nc.tensor.matmul(
                                ps[:, q * 512: q * 512 + NW],
                                lhsT=hT_sb[k][:, m * 128:(m + 1) * 128],
                                rhs=wT_sb[k][:, n * NW:(n + 1) * NW],
                                start=(k == 0),
                                stop=(k == KT - 1) and not with_bias)
                    if with_bias:
                        for q in range(4):
                            n = half * 4 + q
                            nc.tensor.matmul(
                                ps[:, q * 512: q * 512 + NW],
                                lhsT=ones_sb[:],
                                rhs=b_sb[:, n * NW:(n + 1) * NW],
                                start=False, stop=True)
                    src = ps[:].rearrange("p (b n) -> p b n", b=4)[:, :, :NW]
                    dst = x_m[:, half * 4 * NW:(half + 1) * 4 * NW].rearrange(
                        "p (b n) -> p b n", b=4)
                    if j == sz - 1 and half == 0:
                        # the chunk's last h0 drain goes to ACT: it frees the
                        # psum buf the following ext matmuls need, without
                        # waiting out the DVE queue
                        nc.scalar.copy(dst, src)
                    else:
                        nc.vector.tensor_copy(dst, src)
                    # prefetch next ext pair's one-hots mid-chunk so the ext
                    # matmuls never stall the in-order PE queue
                    if g >= 1 and half == 0 and j in oh_at:
                        emit_oh(g - 1, oh_at[j])
                # exp from the drained bf16 logits, accumulating the row sum
                nc.scalar.activation(E[:, :VL], x_m[:], AF.Exp,
                                     accum_out=sums_g[:, j:j + 1])
                if j == min(2, sz - 1) and g >= 1:
                    # the previous chunk's gathered sums are back by now
                    emit_out_begin(g - 1)
            # collective staging: store partial sums (ACT just produced them),
            # all-gather on pool, load all per-core blocks back in one DMA
            # (SBUF side stays partition-first; the DRAM side is permuted).
            nc.scalar.dma_start(cc_in[g][:], sums_g[:, :sz])
            nc.gpsimd.collective_compute(
                "AllGather", OP.bypass,
                replica_groups=[core_ids],
                ins=[cc_in[g][:]], outs=[cc_out[g][:]])
            gath_g = gathpool.tile([128, N_CORES * max(CHS)], F32,
                                   tag="gath")
            gaths[g] = gath_g
            nc.sync.dma_start(
                gath_g[:, :N_CORES * sz].rearrange(
                    "p (r c) -> p r c", r=N_CORES),
                cc_out[g].rearrange("r p c -> p r c"))
            # chunk g-1's subtracts + stores run in the ext-pair window,
            # where DVE has slack
            if g >= 1:
                for j in range(CHS[g - 1]):
                    emit_out_one(g - 1, j)

        # ---- ext pair: scatter-add via one-hot matmul ----
        # batch pairs packed into 64-row psum halves; one-hots on DVE (int16
        # iota, 4x mode); Ln(scale=g1m) reads psum directly; clamp on DVE.
        def emit_ext_pair(p):
            ohs = ext_ohs[p]
            st = stpool.tile([128, VE], BF16, tag="st")
            for grp, gw in ((0, 4), (4, 4), (8, 2)):
                ps = pspool.tile([128, 2048], F32, tag="ps")
                for hl in range(2):
                    for bb in range(2):
                        b = 2 * p + bb
                        lhsT = attnT_sb[:, hl * RL + b * BP:
                                        hl * RL + (b + 1) * BP]
                        for q in range(gw):
                            nb = grp + q
                            nc.tensor.matmul(
                                ps[bb * BP:(bb + 1) * BP,
                                   q * 512: q * 512 + NW],
                                lhsT=lhsT,
                                rhs=ohs[bb][:, nb * NW:(nb + 1) * NW],
                                start=(hl == 0), stop=(hl == 1))
                src = ps[:].rearrange("p (b n) -> p b n", b=4)[:, :gw, :NW]
                dst = st[:, grp * NW:(grp + gw) * NW].rearrange(
                    "p (b n) -> p b n", b=gw)
                # ext = raw * g1m folded into Ln's per-partition scale;
                # clip is done in log space (Ln(0) = -inf clips to LOG_LO)
                nc.scalar.activation(dst, src, AF.Ln, scale=g1m[:, p:p + 1])
            nc.vector.tensor_scalar(st[:], st[:], LOG_LO, LOG_HI,
                                    OP.max, OP.min)
            nc.gpsimd.dma_start(out_ext[2 * p], st[:T, :])
            nc.gpsimd.dma_start(out_ext[2 * p + 1], st[BP:BP + T, :])

        for g in range(NCH):
            emit_chunk(g)
            if g == 0:
                emit_gate()
            else:
                emit_ext_pair(g - 1)
        # last chunk's output tail: half-tile pieces pipeline the final DMAs
        emit_out_begin(NCH - 1)
        for j in range(CHS[NCH - 1]):
            emit_out_one(NCH - 1, j, split=2)

    nc.compile()
    _dedupe_act_table_loads(nc)
    return nc


def _get_nc(with_bias=False):
    key = ("nc", with_bias)
    if key not in _CACHE:
        _CACHE[key] = _build(with_bias)
    return _CACHE[key]


def kernel(**inputs):
    hidden = np.asarray(inputs["hidden"], dtype=np.float32)
    dec = np.asarray(inputs["dec_rnn_output"], dtype=np.float32)
    attn = np.asarray(inputs["attn"], dtype=np.float32)
    c2e = np.asarray(inputs["copy_to_ext"])
    W = np.asarray(inputs["W"], dtype=np.float32)
    bvec = np.asarray(inputs["b"], dtype=np.float32)
    Wc = np.asarray(inputs["Wc"], dtype=np.float32)
    bc = np.asarray(inputs["bc"], dtype=np.float32)

    with_bias = bool(np.any(bvec))
    bf = ml_dtypes.bfloat16
    hT_np = np.ascontiguousarray(
        hidden.reshape(R, H).T.reshape(KT, 128, R)).astype(bf)
    wcT_np = np.ascontiguousarray(Wc.reshape(1, H).T.reshape(KT, 128, 1))
    bc_np = bc.reshape(1, 1)

    in_maps = []
    for c in range(N_CORES):
        vs = slice(c * VL, (c + 1) * VL)
        bs = slice(c * BL, (c + 1) * BL)
        wT_np = np.ascontiguousarray(W[vs].T.reshape(KT, 128, VL)).astype(bf)
        # dT cols are 64-padded b-major: col b*64 + t
        dpad = np.zeros((BL, BP, H), np.float32)
        dpad[:, :T] = dec[:, bs, :].transpose(1, 0, 2)
        dT_np = np.ascontiguousarray(
            dpad.reshape(RL, H).T.reshape(KT, 128, RL))
        # attnT[s, b*64 + t] = attn[t, c*BL+b, s]; hi/lo bf16 split
        apad = np.zeros((S, BL, BP), np.float32)
        apad[:, :, :T] = attn[:, bs, :].transpose(2, 1, 0)
        at = apad.reshape(S, RL)
        at_hi = at.astype(bf)
        at_lo = (at - at_hi.astype(np.float32)).astype(bf)
        attnT_np = np.ascontiguousarray(np.stack([at_hi, at_lo]))
        idx_np = np.ascontiguousarray(c2e[:, bs]).astype(np.float32)
        m = {"dT": dT_np, "wcT": wcT_np, "bc": bc_np,
             "attnT": attnT_np, "idx": idx_np}
        for k in range(KT):
            m[f"hT{k}"] = np.ascontiguousarray(hT_np[k])
            m[f"wT{k}"] = np.ascontiguousarray(wT_np[k])
        if with_bias:
            m["brow"] = bvec[vs].reshape(1, VL).astype(bf)
        in_maps.append(m)

    nc = _get_nc(with_bias)
    res = run_bass_kernel_spmd(nc, in_maps, core_ids=list(range(N_CORES)))

    out = np.empty((T, B, VT + VE), dtype=np.float32)
    for c in range(N_CORES):
        r = res.results[c]
        out[:, :, c * VL:(c + 1) * VL] = np.asarray(
            r["out_tgt"], dtype=np.float32).reshape(T, B, VL)
        out[:, c * BL:(c + 1) * BL, VT:] = np.asarray(
            r["out_ext"], dtype=np.float32).transpose(1, 0, 2)
    return out
